# revision 8
# baseline (speedup 1.0000x reference)
"""Trainium2 Bass kernel for nn_External_Attention (topk_masking).

Data-parallel over batch rows across 8 NeuronCores. Each core handles
B_loc = 2048 rows: q = relu(z@Wq.T+bq), k = v = relu(mc@Wk.T+bk) computed
locally, u = q@k.T/sqrt(128), attn = softmax(u) (written out), exact
per-row top-10 of u via grouped Max8 machinery, mean of the selected v
rows through fc_out.

Self-contained: hardcodes all shapes; only needs the concourse runtime.
"""

import numpy as np

import concourse.bacc as bacc
import concourse.bass as bass
import concourse.mybir as mybir
from concourse.bass import IndirectOffsetOnAxis
from concourse.bass_utils import run_bass_kernel_spmd
from concourse.masks import make_identity
from concourse.tile import TileContext

AF = mybir.ActivationFunctionType
ALU = mybir.AluOpType
AX = mybir.AxisListType
F32 = mybir.dt.float32
I32 = mybir.dt.int32
U16 = mybir.dt.uint16

B, C, H, K = 16384, 4096, 128, 10
NCORES = 8
BLOC = B // NCORES          # 2048 rows per core
P = 128                     # partitions / rows per tile
SCALE = float(np.power(128.0, 0.5))

GS = 16                     # top-k group size
NG = C // GS                # 256 groups
KG = 12                     # candidate groups kept (>= K, ties margin)
CW = KG * GS                # candidate width 256

# normalize split (columns): gpsimd | scalar(ACT) | vector(DVE)
GP_N = 0
ACT_N = 3072
DVE_N = C - GP_N - ACT_N

_CACHE = {}
# extra kwargs for run_bass_kernel_spmd (e.g. trace=True), set by test harness
RUN_KWARGS = {}


def _build(nt: int):
    """Trace + compile the per-core kernel with nt tiles of 128 rows."""
    nc = bacc.Bacc("TRN2", target_bir_lowering=False, debug=False,
                   num_devices=NCORES)

    zT_d = nc.dram_tensor("zT", [H, nt * P], F32, kind="ExternalInput")
    mcT_d = nc.dram_tensor("mcT", [H, C], F32, kind="ExternalInput")
    wqT_d = nc.dram_tensor("wqT", [H, H], F32, kind="ExternalInput")
    bq_d = nc.dram_tensor("bq", [H, 1], F32, kind="ExternalInput")
    wkT_d = nc.dram_tensor("wkT", [H, H], F32, kind="ExternalInput")
    bk_d = nc.dram_tensor("bk", [H, 1], F32, kind="ExternalInput")
    woT_d = nc.dram_tensor("woT", [H, H], F32, kind="ExternalInput")
    bo_d = nc.dram_tensor("bo", [H, 1], F32, kind="ExternalInput")

    attn_d = [nc.dram_tensor(f"attn{t:02d}", [P, C], F32, kind="ExternalOutput")
              for t in range(nt)]
    outT_d = nc.dram_tensor("outT", [H, nt * P], F32, kind="ExternalOutput")

    v_d = nc.dram_tensor("v_scratch", [C, H], F32, kind="Internal")

    with TileContext(nc) as tc:
        with (
            tc.tile_pool(name="const", bufs=1) as cpool,
            tc.tile_pool(name="big", bufs=1) as bigpool,
            tc.tile_pool(name="s", bufs=2) as spool,
            tc.tile_pool(name="attn", bufs=3) as apool,
            tc.tile_pool(name="work", bufs=3) as wpool,
            tc.tile_pool(name="small", bufs=3) as smpool,
            tc.tile_pool(name="upsum", bufs=2, space="PSUM") as upsum,
            tc.tile_pool(name="spsum", bufs=2, space="PSUM") as spsum,
        ):
            # ---- constants / setup ----
            wq_sb = cpool.tile([H, H], F32, tag="wq")
            nc.sync.dma_start(out=wq_sb[:], in_=wqT_d[:])
            wk_sb = cpool.tile([H, H], F32, tag="wk")
            nc.sync.dma_start(out=wk_sb[:], in_=wkT_d[:])
            wo_sb = cpool.tile([H, H], F32, tag="wo")
            nc.sync.dma_start(out=wo_sb[:], in_=woT_d[:])
            bq_sb = cpool.tile([H, 1], F32, tag="bq")
            nc.sync.dma_start(out=bq_sb[:], in_=bq_d[:])
            bk_sb = cpool.tile([H, 1], F32, tag="bk")
            nc.sync.dma_start(out=bk_sb[:], in_=bk_d[:])
            bo_sb = cpool.tile([H, 1], F32, tag="bo")
            nc.sync.dma_start(out=bo_sb[:], in_=bo_d[:])

            zT_sb = cpool.tile([H, nt * P], F32, tag="zT")
            nc.sync.dma_start(out=zT_sb[:], in_=zT_d[:])
            mcT_sb = cpool.tile([H, C], F32, tag="mcT")
            nc.sync.dma_start(out=mcT_sb[:], in_=mcT_d[:])

            ident = cpool.tile([P, P], F32, tag="ident")
            make_identity(nc, ident[:])

            # kT = relu(Wk.T-matmuls over mcT) : [H, C]
            kT_sb = cpool.tile([H, C], F32, tag="kT")
            for cc in range(C // 512):
                kps = upsum.tile([P, 512], F32, tag="u")
                nc.tensor.matmul(kps[:], lhsT=wk_sb[:],
                                 rhs=mcT_sb[:, cc * 512:(cc + 1) * 512],
                                 start=True, stop=True)
                nc.scalar.activation(kT_sb[:, cc * 512:(cc + 1) * 512], kps[:],
                                     AF.Relu, bias=bk_sb[:])
            # v (natural [C, H]) to DRAM via PE transposes
            for i in range(C // P):
                tps = spsum.tile([P, P], F32, tag="sm")
                nc.tensor.transpose(tps[:], kT_sb[:, i * P:(i + 1) * P],
                                    ident[:])
                vt = smpool.tile([P, P], F32, tag="vt")
                nc.scalar.activation(vt[:], tps[:], AF.Copy)
                nc.sync.dma_start(out=v_d[i * P:(i + 1) * P, :], in_=vt[:])

            inv_scale = float(1.0 / SCALE)

            for t in range(nt):
                # ---- scores ----
                qps = spsum.tile([P, P], F32, tag="sm")
                nc.tensor.matmul(qps[:], lhsT=wq_sb[:],
                                 rhs=zT_sb[:, t * P:(t + 1) * P],
                                 start=True, stop=True)
                qT = smpool.tile([P, P], F32, tag="qT")
                nc.scalar.activation(qT[:], qps[:], AF.Relu, bias=bq_sb[:])

                s_sb = spool.tile([P, C], F32, tag="s")
                den_h = smpool.tile([P, 3], F32, tag="den3")
                bounds = [0, 1536, 3072, 4096]
                for h in range(3):
                    lo, hi = bounds[h], bounds[h + 1]
                    ups = upsum.tile([P, 1536], F32, tag="u")
                    for cc in range((hi - lo) // 512):
                        nc.tensor.matmul(
                            ups[:, cc * 512:(cc + 1) * 512], lhsT=qT[:],
                            rhs=kT_sb[:, lo + cc * 512:lo + (cc + 1) * 512],
                            start=True, stop=True)
                    nc.scalar.activation(s_sb[:, lo:hi],
                                         ups[:, 0:hi - lo], AF.Exp,
                                         scale=inv_scale,
                                         accum_out=den_h[:, h:h + 1])
                den01 = smpool.tile([P, 1], F32, tag="den01")
                nc.vector.tensor_tensor(den01[:], den_h[:, 0:1],
                                        den_h[:, 1:2], op=ALU.add)
                den = smpool.tile([P, 1], F32, tag="den")
                nc.vector.tensor_tensor(den[:], den01[:], den_h[:, 2:3],
                                        op=ALU.add)
                recip = smpool.tile([P, 1], F32, tag="recip")
                nc.vector.reciprocal(recip[:], den[:])

                # ---- full-width exact top-10 (values + indices) ----
                c8a = smpool.tile([P, 8], F32, tag="c8a")
                nc.vector.max(c8a[:], s_sb[:])
                s2 = spool.tile([P, C], F32, tag="s2")
                nc.vector.match_replace(s2[:], c8a[:], s_sb[:], -1e30)
                c8b = smpool.tile([P, 8], F32, tag="c8b")
                nc.vector.max(c8b[:], s2[:])
                lia = smpool.tile([P, 8], U16, tag="lia")
                nc.vector.max_index(lia[:], c8a[:], s_sb[:])
                lib = smpool.tile([P, 8], U16, tag="lib")
                nc.vector.max_index(lib[:], c8b[:], s2[:])
                l10 = smpool.tile([P, K], I32, tag="l10")
                nc.vector.tensor_copy(l10[:, 0:8], lia[:])
                nc.vector.tensor_copy(l10[:, 8:K], lib[:, 0:K - 8])

                # attn normalize (3-way split) + write out
                attn_sb = apool.tile([P, C], F32, tag="attn")
                if GP_N:
                    nc.gpsimd.tensor_scalar(attn_sb[:, 0:GP_N],
                                            s_sb[:, 0:GP_N],
                                            recip[:], None, op0=ALU.mult)
                nc.scalar.activation(attn_sb[:, GP_N:GP_N + ACT_N],
                                     s_sb[:, GP_N:GP_N + ACT_N],
                                     AF.Copy, scale=recip[:])
                nc.vector.tensor_scalar(attn_sb[:, GP_N + ACT_N:],
                                        s_sb[:, GP_N + ACT_N:],
                                        recip[:], None, op0=ALU.mult)
                nc.sync.dma_start(out=attn_d[t][:], in_=attn_sb[:])

                # ---- gather v rows (one blessed indirect per rank) ----
                sel = wpool.tile([P, K, H], F32, tag="sel")
                for r in range(K):
                    nc.gpsimd.indirect_dma_start(
                        out=sel[:, r, :], out_offset=None, in_=v_d[:],
                        in_offset=IndirectOffsetOnAxis(ap=l10[:, r:r + 1],
                                                       axis=0))
                selsum = smpool.tile([P, H], F32, tag="selsum")
                nc.vector.tensor_reduce(
                    selsum[:], sel[:].rearrange("p k h -> p h k"),
                    axis=AX.X, op=ALU.add)

                # transpose selsum, then outT = relu(WoT10-mm + bo)
                sps = spsum.tile([P, P], F32, tag="sm")
                nc.tensor.transpose(sps[:], selsum[:], ident[:])
                selsumT = smpool.tile([P, P], F32, tag="selsumT")
                nc.scalar.activation(selsumT[:], sps[:], AF.Copy)
                ops_ = spsum.tile([P, P], F32, tag="sm")
                nc.tensor.matmul(ops_[:], lhsT=wo_sb[:], rhs=selsumT[:],
                                 start=True, stop=True)
                outT_sb = smpool.tile([P, P], F32, tag="outT")
                nc.scalar.activation(outT_sb[:], ops_[:], AF.Relu,
                                     bias=bo_sb[:])
                nc.sync.dma_start(out=outT_d[:, t * P:(t + 1) * P],
                                  in_=outT_sb[:])

    nc.compile()
    return nc


def _get_nc(nt: int):
    if nt not in _CACHE:
        _CACHE[nt] = _build(nt)
    return _CACHE[nt]


def _host_inputs(z, Wq, bq, Wk, bk, memory_cell, Wo, bo):
    """Marshal full inputs into per-core input maps."""
    z = np.asarray(z, np.float32)
    mc = np.asarray(memory_cell, np.float32).reshape(C, H)
    zT = np.ascontiguousarray(z.T)                       # [H, B]
    mcT = np.ascontiguousarray(mc.T)                     # [H, C]
    wqT = np.ascontiguousarray(np.asarray(Wq, np.float32).T)
    wkT = np.ascontiguousarray(np.asarray(Wk, np.float32).T)
    woT = np.ascontiguousarray((np.asarray(Wo, np.float32) / float(K)).T)
    bq_c = np.ascontiguousarray(np.asarray(bq, np.float32).reshape(H, 1))
    bk_c = np.ascontiguousarray(np.asarray(bk, np.float32).reshape(H, 1))
    bo_c = np.ascontiguousarray(np.asarray(bo, np.float32).reshape(H, 1))
    maps = []
    for c in range(NCORES):
        maps.append({
            "zT": np.ascontiguousarray(zT[:, c * BLOC:(c + 1) * BLOC]),
            "mcT": mcT, "wqT": wqT, "bq": bq_c, "wkT": wkT, "bk": bk_c,
            "woT": woT, "bo": bo_c,
        })
    return maps


def kernel(z, Wq, bq, Wk, bk, memory_cell, Wo, bo, topk):
    assert int(topk) == K
    nt = BLOC // P
    nc = _get_nc(nt)
    maps = _host_inputs(z, Wq, bq, Wk, bk, memory_cell, Wo, bo)
    res = run_bass_kernel_spmd(nc, maps, core_ids=list(range(NCORES)),
                               **RUN_KWARGS)
    kernel.last_result = res
    outs = res.results
    attn = np.concatenate(
        [np.concatenate([outs[c][f"attn{t:02d}"] for t in range(nt)], axis=0)
         for c in range(NCORES)], axis=0)
    out = np.concatenate(
        [np.ascontiguousarray(outs[c]["outT"].T) for c in range(NCORES)],
        axis=0)
    c_loss = np.zeros((), np.float32)
    return out, attn, c_loss


# revision 10
# speedup vs baseline: 1.8888x; 1.8888x over previous
"""Trainium2 Bass kernel for nn_External_Attention (topk_masking).

Data-parallel over batch rows across 8 NeuronCores. Each core handles
B_loc = 2048 rows: q = relu(z@Wq.T+bq), k = v = relu(mc@Wk.T+bk) computed
locally, u = q@k.T/sqrt(128), attn = softmax(u) (written out), exact
per-row top-10 of u via grouped Max8 machinery, mean of the selected v
rows through fc_out.

Self-contained: hardcodes all shapes; only needs the concourse runtime.
"""

import numpy as np

import concourse.bacc as bacc
import concourse.bass as bass
import concourse.mybir as mybir
from concourse.bass import IndirectOffsetOnAxis
from concourse.bass_utils import run_bass_kernel_spmd
from concourse.masks import make_identity
from concourse.tile import TileContext

AF = mybir.ActivationFunctionType
ALU = mybir.AluOpType
AX = mybir.AxisListType
F32 = mybir.dt.float32
I32 = mybir.dt.int32
U16 = mybir.dt.uint16

B, C, H, K = 16384, 4096, 128, 10
NCORES = 8
BLOC = B // NCORES          # 2048 rows per core
P = 128                     # partitions / rows per tile
SCALE = float(np.power(128.0, 0.5))

GS = 16                     # top-k group size
NG = C // GS                # 256 groups
KG = 12                     # candidate groups kept (>= K, ties margin)
CW = KG * GS                # candidate width 256

# normalize split (columns): gpsimd | scalar(ACT) | vector(DVE)
GP_N = 1024
ACT_N = 3072
DVE_N = C - GP_N - ACT_N

_CACHE = {}
# extra kwargs for run_bass_kernel_spmd (e.g. trace=True), set by test harness
RUN_KWARGS = {}


def _build(nt: int):
    """Trace + compile the per-core kernel with nt tiles of 128 rows."""
    nc = bacc.Bacc("TRN2", target_bir_lowering=False, debug=False,
                   num_devices=NCORES)

    zT_d = nc.dram_tensor("zT", [H, nt * P], F32, kind="ExternalInput")
    mcT_d = nc.dram_tensor("mcT", [H, C], F32, kind="ExternalInput")
    wqT_d = nc.dram_tensor("wqT", [H, H], F32, kind="ExternalInput")
    bq_d = nc.dram_tensor("bq", [H, 1], F32, kind="ExternalInput")
    wkT_d = nc.dram_tensor("wkT", [H, H], F32, kind="ExternalInput")
    bk_d = nc.dram_tensor("bk", [H, 1], F32, kind="ExternalInput")
    woT_d = nc.dram_tensor("woT", [H, H], F32, kind="ExternalInput")
    bo_d = nc.dram_tensor("bo", [H, 1], F32, kind="ExternalInput")

    attn_d = [nc.dram_tensor(f"attn{t:02d}", [P, C], F32, kind="ExternalOutput")
              for t in range(nt)]
    outT_d = nc.dram_tensor("outT", [H, nt * P], F32, kind="ExternalOutput")

    v_d = nc.dram_tensor("v_scratch", [C, H], F32, kind="Internal")

    with TileContext(nc) as tc:
        with (
            tc.tile_pool(name="const", bufs=1) as cpool,
            tc.tile_pool(name="big", bufs=1) as bigpool,
            tc.tile_pool(name="s", bufs=2) as spool,
            tc.tile_pool(name="attn", bufs=3) as apool,
            tc.tile_pool(name="work", bufs=3) as wpool,
            tc.tile_pool(name="small", bufs=3) as smpool,
            tc.tile_pool(name="upsum", bufs=2, space="PSUM") as upsum,
            tc.tile_pool(name="spsum", bufs=2, space="PSUM") as spsum,
        ):
            # ---- constants / setup ----
            wq_sb = cpool.tile([H, H], F32, tag="wq")
            nc.sync.dma_start(out=wq_sb[:], in_=wqT_d[:])
            wk_sb = cpool.tile([H, H], F32, tag="wk")
            nc.sync.dma_start(out=wk_sb[:], in_=wkT_d[:])
            wo_sb = cpool.tile([H, H], F32, tag="wo")
            nc.sync.dma_start(out=wo_sb[:], in_=woT_d[:])
            bq_sb = cpool.tile([H, 1], F32, tag="bq")
            nc.sync.dma_start(out=bq_sb[:], in_=bq_d[:])
            bk_sb = cpool.tile([H, 1], F32, tag="bk")
            nc.sync.dma_start(out=bk_sb[:], in_=bk_d[:])
            bo_sb = cpool.tile([H, 1], F32, tag="bo")
            nc.sync.dma_start(out=bo_sb[:], in_=bo_d[:])

            zT_sb = cpool.tile([H, nt * P], F32, tag="zT")
            nc.sync.dma_start(out=zT_sb[:], in_=zT_d[:])
            mcT_sb = cpool.tile([H, C], F32, tag="mcT")
            nc.sync.dma_start(out=mcT_sb[:], in_=mcT_d[:])

            ident = cpool.tile([P, P], F32, tag="ident")
            make_identity(nc, ident[:])

            # kT = relu(Wk.T-matmuls over mcT) : [H, C]
            kT_sb = cpool.tile([H, C], F32, tag="kT")
            for cc in range(C // 512):
                kps = upsum.tile([P, 512], F32, tag="u")
                nc.tensor.matmul(kps[:], lhsT=wk_sb[:],
                                 rhs=mcT_sb[:, cc * 512:(cc + 1) * 512],
                                 start=True, stop=True)
                nc.scalar.activation(kT_sb[:, cc * 512:(cc + 1) * 512], kps[:],
                                     AF.Relu, bias=bk_sb[:])
            # v (natural [C, H]) to DRAM via PE transposes
            for i in range(C // P):
                tps = spsum.tile([P, P], F32, tag="sm")
                nc.tensor.transpose(tps[:], kT_sb[:, i * P:(i + 1) * P],
                                    ident[:])
                vt = smpool.tile([P, P], F32, tag="vt")
                nc.scalar.activation(vt[:], tps[:], AF.Copy)
                nc.sync.dma_start(out=v_d[i * P:(i + 1) * P, :], in_=vt[:])

            inv_scale = float(1.0 / SCALE)

            for t in range(nt):
                # ---- scores ----
                qps = spsum.tile([P, P], F32, tag="qps")
                nc.tensor.matmul(qps[:], lhsT=wq_sb[:],
                                 rhs=zT_sb[:, t * P:(t + 1) * P],
                                 start=True, stop=True)
                qT = smpool.tile([P, P], F32, tag="qT")
                nc.scalar.activation(qT[:], qps[:], AF.Relu, bias=bq_sb[:])

                s_sb = spool.tile([P, C], F32, tag="s")
                den_h = smpool.tile([P, 4], F32, tag="den4")
                for h in range(4):
                    lo, hi = h * 1024, (h + 1) * 1024
                    ups = upsum.tile([P, 1024], F32, tag="u")
                    for cc in range(2):
                        nc.tensor.matmul(
                            ups[:, cc * 512:(cc + 1) * 512], lhsT=qT[:],
                            rhs=kT_sb[:, lo + cc * 512:lo + (cc + 1) * 512],
                            start=True, stop=True)
                    nc.scalar.activation(s_sb[:, lo:hi],
                                         ups[:], AF.Exp,
                                         scale=inv_scale,
                                         accum_out=den_h[:, h:h + 1])
                den01 = smpool.tile([P, 1], F32, tag="den01")
                nc.vector.tensor_tensor(den01[:], den_h[:, 0:1],
                                        den_h[:, 1:2], op=ALU.add)
                den23 = smpool.tile([P, 1], F32, tag="den23")
                nc.vector.tensor_tensor(den23[:], den_h[:, 2:3],
                                        den_h[:, 3:4], op=ALU.add)
                den = smpool.tile([P, 1], F32, tag="den")
                nc.vector.tensor_tensor(den[:], den01[:], den23[:],
                                        op=ALU.add)
                recip = smpool.tile([P, 1], F32, tag="recip")
                nc.vector.reciprocal(recip[:], den[:])

                # ---- full-width exact top-10 (values + indices) ----
                c8a = smpool.tile([P, 8], F32, tag="c8a")
                nc.vector.max(c8a[:], s_sb[:])
                s2 = spool.tile([P, C], F32, tag="s2")
                nc.vector.match_replace(s2[:], c8a[:], s_sb[:], -1e30)
                c8b = smpool.tile([P, 8], F32, tag="c8b")
                nc.vector.max(c8b[:], s2[:])
                lia = smpool.tile([P, 8], U16, tag="lia")
                nc.vector.max_index(lia[:], c8a[:], s_sb[:])
                lib = smpool.tile([P, 8], U16, tag="lib")
                nc.vector.max_index(lib[:], c8b[:], s2[:])
                l10 = smpool.tile([P, K], I32, tag="l10")
                nc.vector.tensor_copy(l10[:, 0:8], lia[:])
                nc.vector.tensor_copy(l10[:, 8:K], lib[:, 0:K - 8])

                # attn normalize (3-way split) + write out
                attn_sb = apool.tile([P, C], F32, tag="attn")
                if GP_N:
                    nc.gpsimd.tensor_scalar(attn_sb[:, 0:GP_N],
                                            s_sb[:, 0:GP_N],
                                            recip[:], None, op0=ALU.mult)
                nc.scalar.activation(attn_sb[:, GP_N:GP_N + ACT_N],
                                     s_sb[:, GP_N:GP_N + ACT_N],
                                     AF.Copy, scale=recip[:])
                if DVE_N:
                    nc.vector.tensor_scalar(attn_sb[:, GP_N + ACT_N:],
                                            s_sb[:, GP_N + ACT_N:],
                                            recip[:], None, op0=ALU.mult)
                nc.sync.dma_start(out=attn_d[t][:], in_=attn_sb[:])

                # ---- gather v rows (one blessed indirect per rank) ----
                sel = wpool.tile([P, K, H], F32, tag="sel")
                for r in range(K):
                    nc.gpsimd.indirect_dma_start(
                        out=sel[:, r, :], out_offset=None, in_=v_d[:],
                        in_offset=IndirectOffsetOnAxis(ap=l10[:, r:r + 1],
                                                       axis=0))
                selsum = smpool.tile([P, H], F32, tag="selsum")
                nc.vector.tensor_reduce(
                    selsum[:], sel[:].rearrange("p k h -> p h k"),
                    axis=AX.X, op=ALU.add)

                # transpose selsum, then outT = relu(WoT10-mm + bo)
                sps = spsum.tile([P, P], F32, tag="sm")
                nc.tensor.transpose(sps[:], selsum[:], ident[:])
                selsumT = smpool.tile([P, P], F32, tag="selsumT")
                nc.scalar.activation(selsumT[:], sps[:], AF.Copy)
                ops_ = spsum.tile([P, P], F32, tag="sm")
                nc.tensor.matmul(ops_[:], lhsT=wo_sb[:], rhs=selsumT[:],
                                 start=True, stop=True)
                outT_sb = smpool.tile([P, P], F32, tag="outT")
                nc.scalar.activation(outT_sb[:], ops_[:], AF.Relu,
                                     bias=bo_sb[:])
                nc.sync.dma_start(out=outT_d[:, t * P:(t + 1) * P],
                                  in_=outT_sb[:])

    nc.compile()
    return nc


def _get_nc(nt: int):
    if nt not in _CACHE:
        _CACHE[nt] = _build(nt)
    return _CACHE[nt]


def _host_inputs(z, Wq, bq, Wk, bk, memory_cell, Wo, bo):
    """Marshal full inputs into per-core input maps."""
    z = np.asarray(z, np.float32)
    mc = np.asarray(memory_cell, np.float32).reshape(C, H)
    zT = np.ascontiguousarray(z.T)                       # [H, B]
    mcT = np.ascontiguousarray(mc.T)                     # [H, C]
    wqT = np.ascontiguousarray(np.asarray(Wq, np.float32).T)
    wkT = np.ascontiguousarray(np.asarray(Wk, np.float32).T)
    woT = np.ascontiguousarray((np.asarray(Wo, np.float32) / float(K)).T)
    bq_c = np.ascontiguousarray(np.asarray(bq, np.float32).reshape(H, 1))
    bk_c = np.ascontiguousarray(np.asarray(bk, np.float32).reshape(H, 1))
    bo_c = np.ascontiguousarray(np.asarray(bo, np.float32).reshape(H, 1))
    maps = []
    for c in range(NCORES):
        maps.append({
            "zT": np.ascontiguousarray(zT[:, c * BLOC:(c + 1) * BLOC]),
            "mcT": mcT, "wqT": wqT, "bq": bq_c, "wkT": wkT, "bk": bk_c,
            "woT": woT, "bo": bo_c,
        })
    return maps


def kernel(z, Wq, bq, Wk, bk, memory_cell, Wo, bo, topk):
    assert int(topk) == K
    nt = BLOC // P
    nc = _get_nc(nt)
    maps = _host_inputs(z, Wq, bq, Wk, bk, memory_cell, Wo, bo)
    res = run_bass_kernel_spmd(nc, maps, core_ids=list(range(NCORES)),
                               **RUN_KWARGS)
    kernel.last_result = res
    outs = res.results
    attn = np.concatenate(
        [np.concatenate([outs[c][f"attn{t:02d}"] for t in range(nt)], axis=0)
         for c in range(NCORES)], axis=0)
    out = np.concatenate(
        [np.ascontiguousarray(outs[c]["outT"].T) for c in range(NCORES)],
        axis=0)
    c_loss = np.zeros((), np.float32)
    return out, attn, c_loss
